# revision 9
# baseline (speedup 1.0000x reference)
"""Trainium2 Bass kernel v3 for nn_Attention_local — bf16, engine-balanced,
phase-overlapped.

Data-parallel over batch: 8 images -> 8 NeuronCores, no collectives.

Layout: phase-major padded conv buffer. cv[c, b*BLK + r*33 + (1+w1)] with
b = 4*fy+fx, r = 1+h1 (rows 0/33 and col 0 are zero pads; the left pad of
row r+1 doubles as the right pad of row r). All 9 depthwise taps are
strided reads. d-order per head: d = c_local*16 + (fy*4+fx); n = h1*32+w1
(consistent permutation of the reference layout; projection inverts it).

Chunk order v,v,q,q/k,k so attention (which needs q,k transposed first)
can overlap the tail; x is freed after the last conv matmul so the
attention working set fits in SBUF alongside the tail of phase 1.
"""

import numpy as np
import os

CONV_CP = os.environ.get("K3_CONV_CP", "AV")
STG_CP = os.environ.get("K3_STG_CP", "AV")


def _runs(eng):
    "maximal fx-runs of same engine per fy row: [(eng, fy, fx0, gw)]"
    out = []
    for fy in range(4):
        fx = 0
        while fx < 4:
            e = eng[fy * 4 + fx]
            g = fx
            while g < 4 and eng[fy * 4 + g] == e:
                g += 1
            out.append((e, fy, fx, g - fx))
            fx = g
    return out

HEADS = 4
C = 192
CO = 576
HW = 128
NPIX = HW * HW
NP = 1024
PH = 16
DH = 48 * PH            # 768
BLK = 34 * 33           # 1122 (unused, full-image variant)
HBLK = 18 * 33          # 594: half-image block (16 h1 + 2 halo rows)
CVH = PH * HBLK + 1     # 9505
EPS = 1e-12

# chunk table: (cnt, pieces). piece = (tens, r0, pc, c0) tens:0=q,1=k,2=v
CHUNKS = [
    (128, [(0, 0, 128, 0)]),
    (128, [(0, 0, 64, 128), (1, 64, 64, 0)]),
    (128, [(1, 0, 128, 64)]),
    (128, [(2, 0, 128, 0)]),
    (64, [(2, 0, 64, 128)]),
]
ORDER = [3, 4, 0, 1, 2]

# per-phase engine for dw taps: P=PE diag matmul, V=DVE ts+tt, L=Pool stt
PHASE_ENG = os.environ.get("K3_PH", "PPPPPPPPPPVVVVVV")

_COMPILED = {}


def _build():
    import concourse.bass as bass
    import concourse.bacc as bacc
    import concourse.mybir as mybir
    from concourse.tile import TileContext
    from concourse.masks import make_identity
    from contextlib import ExitStack

    F32 = mybir.dt.float32
    BF16 = mybir.dt.bfloat16
    F8 = mybir.dt.float8e4
    DR = mybir.MatmulPerfMode.DoubleRow
    AF = mybir.ActivationFunctionType
    ALU = mybir.AluOpType

    nc = bacc.Bacc("TRN2", target_bir_lowering=False, debug=False)

    x_d = nc.dram_tensor("x", [C, NPIX], F32, kind="ExternalInput")
    wT_d = nc.dram_tensor("wT", [C, CO], F32, kind="ExternalInput")
    qbr_d = nc.dram_tensor("qbr", [1, CO], F32, kind="ExternalInput")
    aux_d = nc.dram_tensor("aux", [CO, 11], F32, kind="ExternalInput")
    pT_d = nc.dram_tensor("pT", [C, C], F32, kind="ExternalInput")
    pbr_d = nc.dram_tensor("pbr", [1, C], F32, kind="ExternalInput")
    y_d = nc.dram_tensor("y", [C, NPIX], BF16, kind="ExternalOutput")

    TAPS = [(dy, dx) for dy in (-1, 0, 1) for dx in (-1, 0, 1)]

    def tap_off(fy, fx, dy, dx):
        fyp = fy + dy
        cy = 1 if fyp > 3 else (-1 if fyp < 0 else 0)
        fyp -= 4 * cy
        fxp = fx + dx
        cx = 1 if fxp > 3 else (-1 if fxp < 0 else 0)
        fxp -= 4 * cx
        return (4 * fyp + fxp) * HBLK + (1 + cy) * 33 + (1 + cx)

    with TileContext(nc) as tc:
        with ExitStack() as es_all:
            dram = es_all.enter_context(tc.tile_pool(name="dram", bufs=1, space="DRAM"))
            qt_d = dram.tile([1024, C * PH], F8, tag="qt_d")
            kt_d = dram.tile([1024, C * PH], F8, tag="kt_d")
            v_d = dram.tile([HEADS * DH, NP], BF16, tag="v_d")
            o_d = dram.tile([HEADS * DH, NP], BF16, tag="o_d")
            kn_d = dram.tile([C, PH], F32, tag="kn_d")

            es1 = ExitStack()
            es2 = ExitStack()

            # ---------------- phase 1 ----------------
            const = es1.enter_context(tc.tile_pool(name="const", bufs=1))
            identb = const.tile([128, 128], BF16, tag="identb")
            make_identity(nc, identb)
            wqa = const.tile([128, CO], BF16, tag="wqa")
            wqb = const.tile([65, CO], BF16, tag="wqb")

            bigcv = es1.enter_context(tc.tile_pool(name="bigcv", bufs=1))
            big = es1.enter_context(tc.tile_pool(name="big", bufs=2))
            wpool = es1.enter_context(tc.tile_pool(name="wp", bufs=3))
            dgpool = es1.enter_context(tc.tile_pool(name="dg", bufs=2))
            tmpool = es1.enter_context(tc.tile_pool(name="tm", bufs=1))
            tppool = es1.enter_context(tc.tile_pool(name="tmp1", bufs=1))
            tmpool2 = es1.enter_context(tc.tile_pool(name="tmpm", bufs=2))
            npool = es1.enter_context(tc.tile_pool(name="np", bufs=3))
            spool = es1.enter_context(tc.tile_pool(name="sp", bufs=2))
            cps = es1.enter_context(tc.tile_pool(name="cps", bufs=3, space="PSUM"))
            dps = es1.enter_context(tc.tile_pool(name="dps", bufs=3, space="PSUM"))
            tps = es1.enter_context(tc.tile_pool(name="tps", bufs=2, space="PSUM"))

            xpool = es1.enter_context(tc.tile_pool(name="xp", bufs=1))
            xba = xpool.tile([128, NPIX], BF16, tag="xba")
            xbb = xpool.tile([65, NPIX], BF16, tag="xbb")
            with ExitStack() as es0:
                xtmp = es0.enter_context(tc.tile_pool(name="xtmp", bufs=2))
                engs = [nc.scalar.copy,
                        lambda d, s: nc.gpsimd.tensor_copy(d, s),
                        lambda d, s: nc.gpsimd.tensor_copy(d, s)]
                nc.gpsimd.memset(xbb[64:65, :], 1.0)
                wtmp = xtmp.tile([128, CO], F32, tag="wtmp")
                nc.sync.dma_start(wtmp[:], wT_d.ap()[0:128, :])
                nc.scalar.copy(wqa[:], wtmp[:])
                wtmp2 = xtmp.tile([65, CO], F32, tag="wtmp2")
                nc.sync.dma_start(wtmp2[0:64, :], wT_d.ap()[128:192, :])
                nc.sync.dma_start(wtmp2[64:65, :], qbr_d.ap()[0:1, :])
                nc.scalar.copy(wqb[:], wtmp2[:])
                for i in range(16):
                    xf = xtmp.tile([128, 1024], F32, tag="xf")
                    nc.sync.dma_start(
                        xf[:], x_d.ap()[0:128, i * 1024:(i + 1) * 1024])
                    engs[i % 3](xba[:, i * 1024:(i + 1) * 1024], xf[:])
                    xf = xtmp.tile([128, 1024], F32, tag="xf")
                    nc.sync.dma_start(
                        xf[0:64, :], x_d.ap()[128:192, i * 1024:(i + 1) * 1024])
                    engs[(i + 1) % 3](xbb[0:64, i * 1024:(i + 1) * 1024],
                                      xf[0:64, :])

            cva = bigcv.tile([128, CVH], BF16, tag="cva")
            cvb = bigcv.tile([128, CVH], BF16, tag="cvb")
            for cvh, halo in ((cva, 0), (cvb, 17)):
                vw = cvh[:, 0:PH * HBLK].rearrange(
                    "c (b r w) -> c b r w", b=PH, r=18)
                nc.gpsimd.memset(vw[:, :, halo:halo + 1, :], 0.0)
                nc.gpsimd.memset(vw[:, :, :, 0:1], 0.0)
                nc.gpsimd.memset(cvh[:, PH * HBLK:CVH], 0.0)

            def emit_conv(m):
                cnt, pieces = CHUNKS[m]
                w16 = wpool.tile([128, 11], F32, tag="w16")
                nc.sync.dma_start(w16[0:cnt, :],
                                  aux_d.ap()[m * 128:m * 128 + cnt, :])
                dwt = big.tile([128, 16384], BF16, tag="dwt")

                diagw = dgpool.tile([128, 9 * 128], BF16, tag="diagw")
                for t in range(9):
                    nc.vector.tensor_scalar(
                        diagw[0:cnt, t * 128:t * 128 + cnt],
                        identb[0:cnt, 0:cnt],
                        w16[0:cnt, t:t + 1], None, op0=ALU.mult)

                for t in range(32):
                    ps = cps.tile([128, 512], F32, tag="cps")
                    nc.tensor.matmul(
                        ps[0:cnt, :],
                        wqa[:, m * 128:m * 128 + cnt],
                        xba[:, t * 512:(t + 1) * 512],
                        start=True, stop=False)
                    nc.tensor.matmul(
                        ps[0:cnt, :],
                        wqb[:, m * 128:m * 128 + cnt],
                        xbb[0:65, t * 512:(t + 1) * 512],
                        start=False, stop=True)
                    src = bass.AP(ps.tensor, 0,
                                  [[512, cnt], [128, 4], [4, 32], [1, 4]])
                    dsts = []
                    if t <= 16:
                        dsts.append(bass.AP(
                            cva.tensor, (t + 1) * 33 + 1,
                            [[CVH, cnt], [4 * HBLK, 4], [1, 32], [HBLK, 4]]))
                    if t >= 15:
                        dsts.append(bass.AP(
                            cvb.tensor, (t - 15) * 33 + 1,
                            [[CVH, cnt], [4 * HBLK, 4], [1, 32], [HBLK, 4]]))
                    for di, dst in enumerate(dsts):
                        if (t + di) % 4 == 3:
                            nc.vector.tensor_copy(dst, src)
                        else:
                            nc.scalar.copy(dst, src)
                return (cva, cvb), dwt, w16, diagw

            def emit_dw(m, cv2, dwt, w16, diagw):
                cnt, pieces = CHUNKS[m]
                for half in range(2):
                    cvh = cv2[half]
                    ho = half * 512
                    for (eng, fy, fx0, gw) in _runs(PHASE_ENG):
                        if eng == "M":
                            for fx in range(fx0, fx0 + gw):
                                p = fy * 4 + fx
                                dwd = dwt[0:cnt, p * NP + ho:p * NP + ho + 512]
                                csrc = bass.AP(cvh.tensor, tap_off(fy, fx, 0, 0),
                                               [[CVH, cnt], [33, 16], [1, 32]])
                                nc.gpsimd.tensor_scalar(
                                    dwd, csrc, w16[0:cnt, 4:5], w16[0:cnt, 9:10],
                                    op0=ALU.mult, op1=ALU.add)
                                for ti, (dy, dx) in enumerate(TAPS):
                                    if (dy, dx) == (0, 0):
                                        continue
                                    soff = tap_off(fy, fx, dy, dx)
                                    srcm = bass.AP(cvh.tensor, soff,
                                                   [[CVH, cnt], [33, 16], [1, 32]])
                                    tmm = tmpool2.tile([128, 512], BF16, tag="tmm")
                                    nc.gpsimd.tensor_scalar(
                                        tmm[0:cnt, :], srcm,
                                        w16[0:cnt, ti:ti + 1], None, op0=ALU.mult)
                                    nc.gpsimd.tensor_tensor(
                                        dwd, dwd, tmm[0:cnt, :], op=ALU.add)
                            continue
                        if eng == "P":
                            for fx in range(fx0, fx0 + gw):
                                p = fy * 4 + fx
                                dwd = dwt[0:cnt, p * NP + ho:p * NP + ho + 512]
                                pd = dps.tile([128, 512], F32, tag="dps")
                                for ti, (dy, dx) in enumerate(TAPS):
                                    soff = tap_off(fy, fx, dy, dx)
                                    rhs = bass.AP(cvh.tensor, soff,
                                                  [[CVH, cnt], [33, 16], [1, 32]])
                                    nc.tensor.matmul(
                                        pd[0:cnt, :],
                                        diagw[0:cnt, ti * 128:ti * 128 + cnt],
                                        rhs, start=(ti == 0), stop=(ti == 8))
                                nc.scalar.activation(
                                    dwd, pd[0:cnt, :], AF.Identity,
                                    bias=w16[0:cnt, 9:10])
                            continue
                        # V/L: merged across the fx-run; subruns on x-carry
                        p0 = fy * 4 + fx0
                        dwd = bass.AP(dwt.tensor, p0 * NP + ho,
                                      [[16384, cnt], [NP, gw], [1, 512]])
                        csrc = bass.AP(cvh.tensor, tap_off(fy, fx0, 0, 0),
                                       [[CVH, cnt], [HBLK, gw], [33, 16], [1, 32]])
                        nc.vector.tensor_scalar(
                            dwd, csrc, w16[0:cnt, 4:5], w16[0:cnt, 9:10],
                            op0=ALU.mult, op1=ALU.add)
                        for ti, (dy, dx) in enumerate(TAPS):
                            if (dy, dx) == (0, 0):
                                continue
                            subs = [(fx0, gw)]
                            if dx == 1 and fx0 + gw == 4:
                                subs = ([(fx0, gw - 1), (3, 1)] if gw > 1
                                        else [(3, 1)])
                            elif dx == -1 and fx0 == 0:
                                subs = ([(0, 1), (1, gw - 1)] if gw > 1
                                        else [(0, 1)])
                            for (sf, sw) in subs:
                                soff = tap_off(fy, sf, dy, dx)
                                src = bass.AP(cvh.tensor, soff,
                                              [[CVH, cnt], [HBLK, sw],
                                               [33, 16], [1, 32]])
                                dsub = bass.AP(
                                    dwt.tensor, (fy * 4 + sf) * NP + ho,
                                    [[16384, cnt], [NP, sw], [1, 512]])
                                if eng == "V":
                                    tmp = tppool.tile([128, 4096], BF16,
                                                      tag="tmp")
                                    tv = bass.AP(tmp.tensor, 0,
                                                 [[4096, cnt], [1, sw * 512]])
                                    nc.vector.tensor_scalar(
                                        tv, src, w16[0:cnt, ti:ti + 1],
                                        None, op0=ALU.mult)
                                    nc.vector.tensor_tensor(
                                        dsub, dsub, tv, op=ALU.add)
                                else:
                                    nc.gpsimd.scalar_tensor_tensor(
                                        dsub, src, w16[0:cnt, ti:ti + 1],
                                        dsub, op0=ALU.mult, op1=ALU.add)

                # norms / q scale / kn
                for (tens, r0, pc, c0) in pieces:
                    if tens == 2:
                        continue
                    n2 = npool.tile([128, PH], F32, tag="n2")
                    junk = tmpool.tile([128, NP], BF16, tag="junk")
                    for p in range(PH):
                        dv = dwt[r0:r0 + pc, p * NP:(p + 1) * NP]
                        nc.scalar.activation(
                            junk[r0:r0 + pc, :], dv, AF.Square,
                            accum_out=n2[r0:r0 + pc, p:p + 1])
                    nc.scalar.sqrt(n2[r0:r0 + pc, :], n2[r0:r0 + pc, :])
                    nc.vector.tensor_scalar_max(
                        n2[r0:r0 + pc, :], n2[r0:r0 + pc, :], EPS)
                    nc.vector.reciprocal(n2[r0:r0 + pc, :], n2[r0:r0 + pc, :])
                    if tens == 0:
                        nc.vector.tensor_scalar_mul(
                            n2[r0:r0 + pc, :], n2[r0:r0 + pc, :],
                            w16[r0:r0 + pc, 10:11])
                        for p in range(PH):
                            dv = dwt[r0:r0 + pc, p * NP:(p + 1) * NP]
                            nc.gpsimd.tensor_scalar_mul(
                                dv, dv, n2[r0:r0 + pc, p:p + 1])
                    else:
                        nc.vector.tensor_scalar_mul(
                            n2[r0:r0 + pc, :], n2[r0:r0 + pc, :], 1.0 / 256.0)
                        nc.sync.dma_start(kn_d[c0:c0 + pc, :],
                                          n2[r0:r0 + pc, :])

            def emit_stage(m, dwt):
                cnt, pieces = CHUNKS[m]
                if any(t != 2 for (t, _, _, _) in pieces):
                    for nck in range(8):
                        stg = spool.tile([128, 2048], F8, tag="stg")
                        stgv = stg[:].rearrange("n (c q) -> n q c", q=PH)
                        for quad in range(4):
                            tp = tps.tile([128, 512], BF16, tag="tp")
                            for qq in range(4):
                                p = quad * 4 + qq
                                tsrc = bass.AP(
                                    dwt.tensor, p * NP + nck * 128,
                                    [[16384, cnt], [1, 128]])
                                nc.tensor.transpose(
                                    tp[:, qq * 128:qq * 128 + cnt],
                                    tsrc, identb[0:cnt, 0:cnt])
                            srcv = tp[:].rearrange(
                                "n (q c) -> n q c", q=4)[:, :, 0:cnt]
                            dstv = stgv[:, quad * 4:(quad + 1) * 4, 0:cnt]
                            if (quad % 2 == 1) if STG_CP == "AV" else (quad == 2):
                                nc.vector.tensor_scalar(dstv, srcv, 16.0, None,
                                                        op0=ALU.mult)
                            else:
                                nc.scalar.mul(dstv, srcv, 16.0)
                        for (tens, r0, pc, c0) in pieces:
                            if tens == 2:
                                continue
                            tgt = qt_d if tens == 0 else kt_d
                            nc.sync.dma_start(
                                tgt[nck * 128:(nck + 1) * 128,
                                    c0 * 16:(c0 + pc) * 16],
                                stg[:, r0 * 16:(r0 + pc) * 16])
                for (tens, r0, pc, c0) in pieces:
                    if tens != 2:
                        continue
                    vv = v_d[:].rearrange("(a p) n -> a p n", p=PH)
                    src = bass.AP(dwt.tensor, r0 * 16384,
                                  [[16384, pc], [1, 16384]])
                    nc.sync.dma_start(vv[c0:c0 + pc, :, :], src)

            pend = []
            for mi, m in enumerate(ORDER):
                cv, dwt, w16, diagw = emit_conv(m)
                if pend:
                    emit_stage(*pend.pop(0))
                emit_dw(m, cv, dwt, w16, diagw)
                pend.append((m, dwt))
            while pend:
                emit_stage(*pend.pop(0))
            es1.close()

            # ---------------- phase 2: attention ----------------
            vpool = es2.enter_context(tc.tile_pool(name="vp", bufs=1))
            qkp = es2.enter_context(tc.tile_pool(name="qkp", bufs=1))
            epool = es2.enter_context(tc.tile_pool(name="ep", bufs=8))
            knpool = es2.enter_context(tc.tile_pool(name="knp", bufs=3))
            odiv = es2.enter_context(tc.tile_pool(name="od", bufs=4))
            aps = es2.enter_context(tc.tile_pool(name="aps", bufs=2, space="PSUM"))
            ops = es2.enter_context(tc.tile_pool(name="ops", bufs=2, space="PSUM"))
            ones1 = qkp.tile([128, 8], BF16, tag="ones1")
            nc.gpsimd.memset(ones1[:], 1.0)
            knf = kn_d[:].rearrange("a b -> (a b)")

            qts, kts = [], []
            for np_ in range(4):
                qt = qkp.tile([128, 2 * C * PH], F8, tag=f"qt{np_}")
                kt = qkp.tile([128, 2 * C * PH], F8, tag=f"kt{np_}")
                for sub in range(2):
                    r = (np_ * 2 + sub) * 128
                    nc.sync.dma_start(qt[:, sub * 3072:(sub + 1) * 3072],
                                      qt_d[r:r + 128, :])
                    nc.sync.dma_start(kt[:, sub * 3072:(sub + 1) * 3072],
                                      kt_d[r:r + 128, :])
                qts.append(qt)
                kts.append(kt)
            vts = []
            for vc in range(24):
                vt = vpool.tile([128, NP], BF16, tag=f"vt{vc}")
                nc.sync.dma_start(vt[:], v_d[vc * 128:(vc + 1) * 128, :])
                vts.append(vt)

            for h in range(HEADS):
                kn6 = knpool.tile([128, 8], F32, tag="kn")
                k0 = 48 * h * PH
                nc.sync.dma_start(
                    kn6[:, 0:6],
                    bass.AP(kn_d.tensor, kn_d.offset + k0,
                            [[1, 128], [128, 6]]))
                ets = []
                for ec in range(6):
                    kn = kn6[:, ec:ec + 1]
                    pa = aps.tile([128, DH], F32, tag="pa")
                    for np_ in range(4):
                        st, sp = np_ == 0, np_ == 3
                        lhs = bass.AP(kts[np_].tensor,
                                      h * DH + ec * 128,
                                      [[6144, 128], [3072, 2], [1, 128]])
                        rhs0 = bass.AP(qts[np_].tensor, h * DH,
                                       [[6144, 128], [3072, 2], [1, 512]])
                        rhs1 = bass.AP(qts[np_].tensor, h * DH + 512,
                                       [[6144, 128], [3072, 2], [1, 256]])
                        nc.tensor.matmul(pa[:, 0:512], lhs, rhs0,
                                         start=st, stop=sp, perf_mode=DR)
                        nc.tensor.matmul(pa[:, 512:DH], lhs, rhs1,
                                         start=st, stop=sp, perf_mode=DR)
                    et = epool.tile([128, DH], BF16, tag="et")
                    nc.scalar.activation(et[:], pa[:], AF.Exp, scale=kn)
                    ets.append(et)
                for dc in range(6):
                    po = ops.tile([128, 1024], F32, tag="po")
                    zp = aps.tile([128, DH], F32, tag="pa")
                    for ec in range(6):
                        st, sp = ec == 0, ec == 5
                        lhs = ets[ec][:, dc * 128:(dc + 1) * 128]
                        vb = vts[h * 6 + ec]
                        nc.tensor.matmul(po[:, 0:512], lhs, vb[:, 0:512],
                                         start=st, stop=sp)
                        nc.tensor.matmul(po[:, 512:1024], lhs, vb[:, 512:1024],
                                         start=st, stop=sp)
                        nc.tensor.matmul(zp[:, 0:8], lhs, ones1[:],
                                         start=st, stop=sp)
                    zr = odiv.tile([128, 1], F32, tag="zr")
                    nc.vector.tensor_scalar_add(zr[:], zp[:, 0:1], 1.0)
                    nc.vector.reciprocal(zr[:], zr[:])
                    ot = odiv.tile([128, NP], BF16, tag="ot")
                    if dc % 2 == 0:
                        nc.scalar.mul(ot[:], po[:, 0:1024], zr[:])
                    else:
                        nc.vector.tensor_scalar_mul(ot[:], po[:, 0:1024], zr[:])
                    nc.sync.dma_start(
                        o_d[h * DH + dc * 128:h * DH + (dc + 1) * 128, :], ot[:])
            es2.close()

            # ---------------- phase 3: projection ----------------
            with ExitStack() as es3:
                ppool = es3.enter_context(tc.tile_pool(name="pp", bufs=1))
                pps = es3.enter_context(tc.tile_pool(name="pps", bufs=4, space="PSUM"))
                ptmp = ppool.tile([128, C], F32, tag="ptmp")
                nc.sync.dma_start(ptmp[:], pT_d.ap()[0:128, :])
                pwa = ppool.tile([128, C], BF16, tag="pwa")
                nc.scalar.copy(pwa[:], ptmp[:])
                ptmp2 = ppool.tile([65, C], F32, tag="ptmp2")
                nc.sync.dma_start(ptmp2[0:64, :], pT_d.ap()[128:192, :])
                nc.sync.dma_start(ptmp2[64:65, :], pbr_d.ap()[0:1, :])
                pwb = ppool.tile([65, C], BF16, tag="pwb")
                nc.scalar.copy(pwb[:], ptmp2[:])

                oa = ppool.tile([128, NPIX], BF16, tag="oa")
                ob = ppool.tile([65, NPIX], BF16, tag="ob")
                ov2 = o_d[:].rearrange("(c r) n -> c (r n)", r=PH)
                for i in range(8):
                    nc.sync.dma_start(oa[:, i * 2048:(i + 1) * 2048],
                                      ov2[0:128, i * 2048:(i + 1) * 2048])
                    nc.sync.dma_start(ob[0:64, i * 2048:(i + 1) * 2048],
                                      ov2[128:192, i * 2048:(i + 1) * 2048])
                nc.gpsimd.memset(ob[64:65, :], 1.0)

                ya = ppool.tile([128, NPIX], BF16, tag="ya")
                yb = ppool.tile([64, NPIX], BF16, tag="yb")
                for nh in range(2):
                    for p in range(PH):
                        fy, fx = p // 4, p % 4
                        for (yt, m0, mc) in ((ya, 0, 128), (yb, 128, 64)):
                            ps = pps.tile([128, 512], F32, tag="pps")
                            nc.tensor.matmul(
                                ps[0:mc, :], pwa[:, m0:m0 + mc],
                                oa[:, p * NP + nh * 512:p * NP + (nh + 1) * 512],
                                start=True, stop=False)
                            nc.tensor.matmul(
                                ps[0:mc, :], pwb[:, m0:m0 + mc],
                                ob[:, p * NP + nh * 512:p * NP + (nh + 1) * 512],
                                start=False, stop=True)
                            dst = bass.AP(
                                yt.tensor,
                                (nh * 64 + fy) * 128 + fx,
                                [[NPIX, mc], [512, 16], [4, 32]])
                            src = bass.AP(ps.tensor, 0,
                                          [[512, mc], [32, 16], [1, 32]])
                            if p % 2 == 0:
                                nc.scalar.copy(dst, src)
                            else:
                                nc.vector.tensor_copy(dst, src)
                    nc.sync.dma_start(
                        y_d.ap()[0:128, nh * 8192:(nh + 1) * 8192],
                        ya[:, nh * 8192:(nh + 1) * 8192])
                    nc.sync.dma_start(
                        y_d.ap()[128:192, nh * 8192:(nh + 1) * 8192],
                        yb[:, nh * 8192:(nh + 1) * 8192])

    nc.compile()
    return nc


def kernel(**inputs):
    import concourse.bass_utils as bu

    x = np.asarray(inputs["x"], np.float32)
    qkv_w = np.asarray(inputs["qkv_w"], np.float32)
    qkv_b = np.asarray(inputs["qkv_b"], np.float32)
    dw_w = np.asarray(inputs["dw_w"], np.float32)
    dw_b = np.asarray(inputs["dw_b"], np.float32)
    proj_w = np.asarray(inputs["proj_w"], np.float32)
    proj_b = np.asarray(inputs["proj_b"], np.float32)
    temp = np.asarray(inputs["temperature"], np.float32).reshape(HEADS)

    if "nc" not in _COMPILED:
        _COMPILED["nc"] = _build()
    nc = _COMPILED["nc"]

    aux = np.zeros((CO, 11), np.float32)
    aux[:, 0:9] = dw_w.reshape(CO, 9)
    aux[:, 9] = dw_b
    aux[:, 10] = np.concatenate([np.repeat(temp, 48)] * 3)

    common = {
        "wT": np.ascontiguousarray(qkv_w.T),
        "qbr": np.ascontiguousarray(qkv_b.reshape(1, CO)),
        "aux": aux,
        "pT": np.ascontiguousarray(proj_w.T),
        "pbr": np.ascontiguousarray(proj_b.reshape(1, C)),
    }
    in_maps = [
        {"x": np.ascontiguousarray(x[b].reshape(C, NPIX)), **common}
        for b in range(x.shape[0])
    ]
    res = bu.run_bass_kernel_spmd(nc, in_maps, core_ids=list(range(len(in_maps))))
    out = np.stack([np.asarray(r["y"], np.float32).reshape(C, HW, HW)
                    for r in res.results])
    return out.astype(np.float32)
